# revision 26
# baseline (speedup 1.0000x reference)
"""Causal self-attention (B=2, T=2048, C=1024, H=16) on 8 Trainium2 cores.

Sharding: tensor-parallel over heads (2 heads/core). Each core computes
QKV projection for its heads, causal attention, and a partial c_proj
output; partials are summed on the host. The v-projection bias and
b_proj fold into the host reduction (softmax weights sum to 1, so the
v-bias contributes the constant vector b_v @ W_proj to every row).

All matmuls run in bf16 (1 PE cycle/row at any free size under the
cost model; rel err budget is 2e-2 and bf16 lands ~1e-3 end to end).

Per-core dataflow, everything K-major so no PE transposes at all:
  xT [C, B*T] bf16 (host pre-transposes x)
  qT/kT [128, B*T] = Wqk^T @ x + b            (PE; epilogue adds bias)
  v     [r 128, 65]  = x^T-stationary matmul  (PE; direct [r, hs] layout,
                                               ones col for softmax sums)
  S^T pair [k 128, 2*512] = K @ Q^T           (PE; causal tiles only)
  E^T = exp(S^T/8) over the flat [128, <=1024] span  (ACT, 2 tiles/inst)
  diag 128x128 blocks masked post-exp         (Pool, 0/1 trimask)
  y'^T [65, 512] += v_aug^T @ E^T             (PE; row 64 = softmax sums)
  rec = 1/y'[64] (DVE, PSUM direct); bcast via rank-1 PE matmul
  ynT = y'[0:64] * bcast                      (DVE)
  partial^T [c 128, 512] = Wp_local^T @ ynT   (PE) -> bf16 -> DRAM

The PE executes in order, so emission order = PE schedule: background
work (next batch's QKV/v projections, c_proj row tiles) is drip-fed
into the attention kc loop one thunk at a time to cover the exp
latency (ACT is slightly slower per tile pair than PE).
"""

import numpy as np
import ml_dtypes

import concourse.bass as bass
import concourse.tile as tile
from concourse import bacc, mybir
from concourse.bass_utils import run_bass_kernel_spmd

F32 = mybir.dt.float32
F32R = mybir.dt.float32r
BF16 = mybir.dt.bfloat16

B, T, C, H = 2, 2048, 1024, 16
HS = C // H            # 64 head dim
NCORES = 8
HL = H // NCORES       # 2 local heads
LC = HL * HS           # 128 local q/k/v cols
R = B * T              # 4096 rows
KC = C // 128          # 8 contraction chunks for projections
QT = 512               # attention q tile
NQT = T // QT          # 4
KA = 128               # attention k chunk
RT = 512               # row tile for projections
NRT = R // RT          # 8
NCC = C // 128         # 8 c_proj output chunks
BF = ml_dtypes.bfloat16


def _flat(t, lo, hi):
    """Contiguous free-dim span [lo, hi) of a tile viewed as [part, hi-lo]."""
    return bass.AP(tensor=t.tensor, offset=t.offset + lo,
                   ap=[list(t.ap[0]), [1, hi - lo]])


def build_program():
    nc = bacc.Bacc("TRN2", target_bir_lowering=False, debug=False,
                   num_devices=NCORES)

    xT = nc.dram_tensor("xT", [C, R], BF16, kind="ExternalInput").ap()
    wqk = nc.dram_tensor("wqk", [C, 2 * LC], BF16, kind="ExternalInput").ap()
    wv = nc.dram_tensor("wv", [C, LC], BF16, kind="ExternalInput").ap()
    bqk = nc.dram_tensor("bqk", [2 * LC], F32, kind="ExternalInput").ap()
    wp = nc.dram_tensor("wp", [LC, C], BF16, kind="ExternalInput").ap()
    trimask = nc.dram_tensor("trimask", [KA, KA], BF16,
                             kind="ExternalInput").ap()
    outT = nc.dram_tensor("outT", [C, R], BF16, kind="ExternalOutput").ap()

    with tile.TileContext(nc) as tc:
        with (
            tc.tile_pool(name="consts", bufs=1) as consts,
            tc.tile_pool(name="weights", bufs=1) as weights,
            tc.tile_pool(name="qkvT", bufs=1) as qkvT_pool,
            tc.tile_pool(name="xs", bufs=NRT) as xs_pool,
            tc.tile_pool(name="vh", bufs=2 * B * T // KA) as vh_pool,
            tc.tile_pool(name="et", bufs=4) as et_pool,
            tc.tile_pool(name="rec", bufs=2) as rec_pool,
            tc.tile_pool(name="osb", bufs=12) as osb_pool,
            tc.tile_pool(name="dscr", bufs=4, space="DRAM") as dscr_pool,
            tc.tile_pool(name="mm512", bufs=2, space="PSUM") as mm512_pool,
            tc.tile_pool(name="ytps", bufs=2, space="PSUM") as ytps_pool,
            tc.tile_pool(name="smps", bufs=2, space="PSUM") as smps_pool,
        ):
            # ---- constants ----
            ones64_f = consts.tile([1, HS], F32)
            nc.vector.memset(ones64_f, 1.0)
            ones64 = consts.tile([1, HS], F32R)
            nc.vector.tensor_copy(ones64, ones64_f)
            tri_sb = consts.tile([KA, KA], BF16)
            bqk_sb = consts.tile([128, 2], F32)

            wqk_sb = weights.tile([128, KC, 2 * LC], BF16)
            wv_sb = weights.tile([128, KC, LC], BF16)
            wp_sb = weights.tile([LC, C], BF16)

            wqk_r = wqk.rearrange("(kc p) n -> p kc n", p=128)
            wv_r = wv.rearrange("(kc p) n -> p kc n", p=128)

            # All x is prefetched up front (no DMA waits inside the
            # attention phase). HWDGE generation (~630ns) and the transfer
            # pipe serialize across DMAs, so keep the COUNT minimal:
            # one DMA per x row tile, whole-tensor weight loads.
            x_tiles = []
            for rt in range(NRT):
                x_sb = xs_pool.tile([128, KC, RT], BF16, tag="xs",
                                    name=f"x_sb{rt}")
                x_tiles.append(x_sb)

            def load_x(rt, lo=0, hi=KC):
                x_r = xT[:, rt * RT:(rt + 1) * RT].rearrange(
                    "(kc p) r -> p kc r", p=128)
                nc.scalar.dma_start(out=x_tiles[rt][:, lo:hi],
                                    in_=x_r[:, lo:hi])

            nc.sync.dma_start(out=wqk_sb[:, 0:2], in_=wqk_r[:, 0:2])
            load_x(0, 0, 4)
            nc.sync.dma_start(
                out=bqk_sb, in_=bqk.rearrange("(j p) -> p j", p=128))
            nc.sync.dma_start(out=wv_sb, in_=wv_r)
            nc.sync.dma_start(out=wqk_sb[:, 2:KC], in_=wqk_r[:, 2:KC])
            load_x(0, 4, KC)
            load_x(1)
            nc.sync.dma_start(out=tri_sb, in_=trimask)
            nc.sync.dma_start(out=wp_sb, in_=wp)
            for rt in range(2, 5):
                load_x(rt)
            # rt5-7 x loads are deferred into the batch-0 loop so the DMA
            # pipe is clear for batch-0's output stores

            qT_s = qkvT_pool.tile([LC, R], BF16, tag="qT")
            kT_s = qkvT_pool.tile([LC, R], BF16, tag="kT")
            ynT_s = qkvT_pool.tile([LC, R], BF16, tag="ynT")

            vh_tiles = {}   # (b, chunk) -> [128, 130] tile (65 per head)
            qkv_done = set()  # row tiles whose QKV thunks have all run
            epi_rr = [0]    # epilogue engine round-robin state

            act_free = [True]  # ACT has slack until exps start / after last

            def psum_to_sbuf(dst, src, bias=None):
                """PSUM->SBUF epilogue: DVE while ACT is exp-bound, 50/50
                DVE/ACT otherwise."""
                i = epi_rr[0]
                epi_rr[0] += 1
                on_act = act_free[0] and (i % 2 == 1)
                if bias is not None:
                    if on_act:
                        nc.scalar.activation(
                            dst, src, mybir.ActivationFunctionType.Identity,
                            bias=bias)
                    else:
                        nc.vector.tensor_scalar_add(dst, src, bias)
                    return
                if on_act:
                    nc.scalar.activation(
                        dst, src, mybir.ActivationFunctionType.Identity)
                else:
                    nc.vector.tensor_copy(dst, src)

            def qk_mm(rt, col, lohi, ps):
                dst = qT_s if col == 0 else kT_s
                for kc in range(*lohi):
                    nc.tensor.matmul(
                        ps,
                        wqk_sb[:, kc, col * LC:(col + 1) * LC],
                        x_tiles[rt][:, kc, :],
                        start=(kc == 0),
                        stop=(kc == KC - 1),
                    )
                if lohi[1] == KC:
                    psum_to_sbuf(dst[:, rt * RT:(rt + 1) * RT], ps,
                                 bias=bqk_sb[:, col:col + 1])

            def v_mm(rt, j, ps):
                """v for row chunk (rt*4+j) directly in [r, hs] layout."""
                for kc in range(KC):
                    nc.tensor.matmul(
                        ps[:, j * KA:(j + 1) * KA],
                        x_tiles[rt][:, kc, j * KA:(j + 1) * KA],
                        wv_sb[:, kc, :],
                        start=(kc == 0),
                        stop=(kc == KC - 1),
                    )

            def v_epi(rt, j, ps):
                chunk = rt * (RT // KA) + j
                b = chunk // (T // KA)
                ch = chunk % (T // KA)
                vh = vh_pool.tile([128, 2 * (HS + 1)], BF16, tag="vh",
                                  name=f"vh_b{b}c{ch}")
                src = ps[:, j * KA:(j + 1) * KA]
                nc.vector.tensor_copy(
                    bass.AP(tensor=vh.tensor, offset=vh.offset,
                            ap=[list(vh.ap[0]), [HS + 1, 2], [1, HS]]),
                    bass.AP(tensor=src.tensor, offset=src.offset,
                            ap=[list(src.ap[0]), [HS, 2], [1, HS]]),
                )
                nc.gpsimd.memset(
                    bass.AP(tensor=vh.tensor, offset=vh.offset + HS,
                            ap=[list(vh.ap[0]), [HS + 1, 2], [1, 1]]),
                    1.0)
                vh_tiles[(b, ch)] = vh

            def qkv_rt_thunks(rt):
                """Thunk list for one 512-row QKV tile (~2.5us of PE)."""
                ps_qk = [None, None]
                ps_v = [None]

                def qk_a(col):
                    def f():
                        ps_qk[col] = mm512_pool.tile(
                            [128, RT], F32, tag="mm512",
                            name=f"qkps{rt}c{col}")
                        qk_mm(rt, col, (0, KC // 2), ps_qk[col])
                    return f

                def qk_b(col):
                    return lambda: qk_mm(rt, col, (KC // 2, KC), ps_qk[col])

                def v_a(j):
                    def f():
                        if j == 0:
                            ps_v[0] = mm512_pool.tile(
                                [128, RT], F32, tag="mm512",
                                name=f"vps{rt}")
                        v_mm(rt, j, ps_v[0])
                    return f

                def v_b(j):
                    return lambda: v_epi(rt, j, ps_v[0])

                out = [qk_a(0), qk_b(0), qk_a(1), qk_b(1)]
                for j in range(RT // KA):
                    out.append(v_a(j))
                for j in range(RT // KA):
                    out.append(v_b(j))
                out.append(lambda: qkv_done.add(rt))
                return out

            o2_state = {}

            def proj_thunk(rt, cc):
                """One c_proj output chunk: matmul + epilogue (+store)."""
                def f():
                    pp = mm512_pool.tile([128, RT], F32, tag="mm512",
                                         name=f"pp{rt}c{cc}")
                    nc.tensor.matmul(
                        pp,
                        wp_sb[:, cc * 128:(cc + 1) * 128],
                        ynT_s[:, rt * RT:(rt + 1) * RT],
                        start=True,
                        stop=True,
                    )
                    if cc % 2 == 0:
                        o2_state[rt] = osb_pool.tile(
                            [128, 2, RT], BF16, tag="osb",
                            name=f"o2_{rt}_{cc}")
                    o2 = o2_state[rt]
                    psum_to_sbuf(o2[:, cc % 2], pp)
                    if cc % 2 == 1:
                        dst = outT[(cc - 1) * 128:(cc + 1) * 128,
                                   rt * RT:(rt + 1) * RT]
                        nc.sync.dma_start(
                            out=dst.rearrange("(g p) r -> p g r", p=128),
                            in_=o2)
                return f

            bg = []

            def bg_step(n=1):
                for _ in range(n):
                    if bg:
                        bg.pop(0)()

            def attn_qt(b, h, qt, tail=False):
                # make sure the QKV thunks this tile reads are all emitted
                need_rt = b * (NRT // 2) + qt
                while need_rt not in qkv_done and bg:
                    bg_step(1)
                base = b * T
                hsl = slice(h * HS, (h + 1) * HS)
                qcols = slice(base + qt * QT, base + (qt + 1) * QT)
                nka = (qt + 1) * (QT // KA)
                pairs = [(kc, kc + 1) for kc in range(0, nka, 2)]
                pend = None

                yp = ytps_pool.tile([HS + 1, QT], F32, tag="yt",
                                    name=f"yt_b{b}h{h}q{qt}")

                def av_pair(info):
                    sps2, et2, p2 = info
                    for i, kc in enumerate(p2):
                        diag = (kc * KA // QT == qt)
                        off = kc * KA - qt * QT if diag else 0
                        nc.tensor.matmul(
                            yp[:, off:QT],
                            vh_tiles[(b, kc)][:, h * (HS + 1):
                                              (h + 1) * (HS + 1)],
                            et2[:, i, off:QT],
                            start=(kc == 0),
                            stop=(kc == nka - 1),
                        )

                for p2 in pairs:
                    sps2 = smps_pool.tile(
                        [128, 2, QT], F32, tag="sm",
                        name=f"sps_b{b}h{h}q{qt}k{p2[0]}")
                    for i, kc in enumerate(p2):
                        nc.tensor.matmul(
                            sps2[:, i],
                            kT_s[hsl, base + kc * KA:base + (kc + 1) * KA],
                            qT_s[hsl, qcols],
                            start=True,
                            stop=True,
                        )
                    et2 = et_pool.tile([128, 2, QT], BF16, tag="et",
                                       name=f"et_b{b}h{h}q{qt}k{p2[0]}")
                    offs = [kc * KA - qt * QT if kc * KA // QT == qt else 0
                            for kc in p2]
                    scale = 1.0 / np.sqrt(HS).item()
                    if offs[1] <= KA:
                        # one flat span; <=128 wasted rows beat a 2nd init
                        nc.scalar.activation(
                            _flat(et2, offs[0], 2 * QT),
                            _flat(sps2, offs[0], 2 * QT),
                            mybir.ActivationFunctionType.Exp, scale=scale)
                    else:
                        for i in range(2):
                            nc.scalar.activation(
                                _flat(et2, i * QT + offs[i], (i + 1) * QT),
                                _flat(sps2, i * QT + offs[i], (i + 1) * QT),
                                mybir.ActivationFunctionType.Exp,
                                scale=scale)
                    for i, kc in enumerate(p2):
                        if kc * KA // QT == qt:
                            # Pool: contention-free (DVE queueing would put
                            # epilogues on the exp->mask->AV chain)
                            o = kc * KA - qt * QT
                            nc.gpsimd.tensor_mul(
                                et2[:, i, o:o + KA], et2[:, i, o:o + KA],
                                tri_sb)
                    if pend is not None:
                        av_pair(pend)
                    pend = (sps2, et2, p2)
                    bg_step(2 if len(bg) > 12 else 1)
                av_pair(pend)

                # softmax normalize. Default: DMA-bounce broadcast of the
                # reciprocal row (no PE/ACT cost, ~3.5us latency hidden by
                # the next qt). Tail: rank-1 PE broadcast + SBUF bounce of
                # y, the shortest-latency chain.
                rec = rec_pool.tile([1, QT], F32R, tag="rec",
                                    name=f"rec_b{b}h{h}q{qt}")
                with nc.allow_low_precision(
                        reason="f32r reciprocal: ~1e-4 rel err ok"):
                    nc.vector.reciprocal(rec, yp[HS:HS + 1, :])
                if tail or b == 0:
                    # batch 0: the x prefetch owns the DMA pipe (FIFO), a
                    # bounce would queue ~20us behind it -- use the PE
                    # rank-1 broadcast instead. Also the tail (latency).
                    yc = rec_pool.tile([HS, QT], F32, tag="yc",
                                       name=f"yc_b{b}h{h}q{qt}")
                    nc.scalar.activation(
                        yc, yp[0:HS, :],
                        mybir.ActivationFunctionType.Identity)
                    bcp = mm512_pool.tile([HS, QT], F32, tag="mm512",
                                          name=f"bcp_b{b}h{h}q{qt}")
                    nc.tensor.matmul(bcp, ones64, rec, start=True, stop=True)
                    nc.vector.tensor_mul(ynT_s[hsl, qcols], yc, bcp)
                else:
                    recd = dscr_pool.tile([1, QT], F32R, tag="recd",
                                          name=f"recd_b{b}h{h}q{qt}")
                    nc.sync.dma_start(out=recd, in_=rec)
                    bcs = rec_pool.tile([HS, QT], F32R, tag="bcs",
                                        name=f"bcs_b{b}h{h}q{qt}")
                    rec_bcast = bass.AP(
                        tensor=recd.tensor, offset=recd.offset,
                        ap=[[0, HS]] + [list(d) for d in recd.ap[1:]])
                    nc.sync.dma_start(out=bcs, in_=rec_bcast)
                    nc.vector.tensor_mul(ynT_s[hsl, qcols], yp[0:HS, :], bcs)
                bg_step(2)

            # ---- symmetric per-batch schedule: first row tile direct,
            # then per qt: both heads' attention with the next row tile's
            # QKV as background filler, proj chasing one qt behind ----
            for b in range(B):
                rt0 = b * (NRT // 2)
                for th in qkv_rt_thunks(rt0):
                    th()
                for qt in range(NQT):
                    if b == 0 and qt < 3:
                        load_x(5 + qt)  # deferred b1 x prefetch
                    if qt + 1 < NQT:
                        bg.extend(qkv_rt_thunks(rt0 + qt + 1))
                    act_free[0] = False
                    attn_qt(b, 0, qt)
                    attn_qt(b, 1, qt, tail=(b == B - 1 and qt == NQT - 1))
                    for cc in range(NCC):
                        bg.append(proj_thunk(rt0 + qt, cc))
            # exps are done: the drain's epilogues can use ACT again
            act_free[0] = True
            while bg:
                bg_step(1)

    nc.compile()
    return nc


_NC = None


def _get_nc():
    global _NC
    if _NC is None:
        _NC = build_program()
    return _NC


def make_in_maps(x, W_attn, b_attn, W_proj, b_proj):
    x = np.asarray(x, np.float32)
    W_attn = np.asarray(W_attn, np.float32)
    b_attn = np.asarray(b_attn, np.float32)
    W_proj = np.asarray(W_proj, np.float32)

    xT = np.ascontiguousarray(x.reshape(R, C).T).astype(BF)
    tri = np.triu(np.ones((KA, KA), np.float32)).astype(BF)

    in_maps = []
    for core in range(NCORES):
        g0 = core * LC
        cols = slice(g0, g0 + LC)
        wqk_l = np.concatenate(
            [W_attn[:, 0:C][:, cols], W_attn[:, C:2 * C][:, cols]], axis=1)
        bqk_l = np.concatenate(
            [b_attn[0:C][cols], b_attn[C:2 * C][cols]])
        in_maps.append({
            "xT": xT,
            "wqk": np.ascontiguousarray(wqk_l).astype(BF),
            "wv": np.ascontiguousarray(
                W_attn[:, 2 * C:3 * C][:, cols]).astype(BF),
            "bqk": np.ascontiguousarray(bqk_l),
            "wp": np.ascontiguousarray(W_proj[cols, :]).astype(BF),
            "trimask": tri,
        })
    return in_maps


def kernel(x, W_attn, b_attn, W_proj, b_proj):
    nc = _get_nc()
    in_maps = make_in_maps(x, W_attn, b_attn, W_proj, b_proj)
    res = run_bass_kernel_spmd(nc, in_maps, list(range(NCORES)))
    acc = res.results[0]["outT"].astype(np.float32)
    for corer in res.results[1:]:
        acc += corer["outT"].astype(np.float32)
    out = np.ascontiguousarray(acc.T).reshape(B, T, C)
    # v-bias and c_proj bias fold into the host-side reduction epilogue:
    # softmax rows sum to 1, so b_v contributes b_v @ W_proj to every row.
    b_attn = np.asarray(b_attn, np.float32)
    out += np.asarray(b_proj, np.float32) + b_attn[2 * C:] @ np.asarray(
        W_proj, np.float32)
    return out


# revision 31
# speedup vs baseline: 1.1071x; 1.1071x over previous
"""Causal self-attention (B=2, T=2048, C=1024, H=16) on 8 Trainium2 cores.

Sharding: tensor-parallel over heads (2 heads/core). Each core computes
QKV projection for its heads, causal attention, and a partial c_proj
output; partials are summed on the host. The v-projection bias and
b_proj fold into the host reduction (softmax weights sum to 1, so the
v-bias contributes the constant vector b_v @ W_proj to every row).

All matmuls run in bf16 (1 PE cycle/row at any free size under the
cost model; rel err budget is 2e-2 and bf16 lands ~1e-3 end to end).

Per-core dataflow, everything K-major so no PE transposes at all:
  xT [C, B*T] bf16 (host pre-transposes x)
  qT/kT [128, B*T] = Wqk^T @ x + b            (PE; epilogue adds bias)
  v     [r 128, 65]  = x^T-stationary matmul  (PE; direct [r, hs] layout,
                                               ones col for softmax sums)
  S^T pair [k 128, 2*512] = K @ Q^T           (PE; causal tiles only)
  E^T = exp(S^T/8) over the flat [128, <=1024] span  (ACT, 2 tiles/inst)
  diag 128x128 blocks masked post-exp         (Pool, 0/1 trimask)
  y'^T [65, 512] += v_aug^T @ E^T             (PE; row 64 = softmax sums)
  rec = 1/y'[64] (DVE, PSUM direct); bcast via rank-1 PE matmul
  ynT = y'[0:64] * bcast                      (DVE)
  partial^T [c 128, 512] = Wp_local^T @ ynT   (PE) -> bf16 -> DRAM

The PE executes in order, so emission order = PE schedule: background
work (next batch's QKV/v projections, c_proj row tiles) is drip-fed
into the attention kc loop one thunk at a time to cover the exp
latency (ACT is slightly slower per tile pair than PE).
"""

import numpy as np
import ml_dtypes

import concourse.bass as bass
import concourse.tile as tile
from concourse import bacc, mybir
from concourse.bass_utils import run_bass_kernel_spmd

F32 = mybir.dt.float32
F32R = mybir.dt.float32r
BF16 = mybir.dt.bfloat16

B, T, C, H = 2, 2048, 1024, 16
HS = C // H            # 64 head dim
NCORES = 8
HL = H // NCORES       # 2 local heads
LC = HL * HS           # 128 local q/k/v cols
R = B * T              # 4096 rows
KC = C // 128          # 8 contraction chunks for projections
QT = 512               # attention q tile
NQT = T // QT          # 4
KA = 128               # attention k chunk
RT = 512               # row tile for projections
NRT = R // RT          # 8
NCC = C // 128         # 8 c_proj output chunks
BF = ml_dtypes.bfloat16


def _flat(t, lo, hi):
    """Contiguous free-dim span [lo, hi) of a tile viewed as [part, hi-lo]."""
    return bass.AP(tensor=t.tensor, offset=t.offset + lo,
                   ap=[list(t.ap[0]), [1, hi - lo]])


def build_program():
    nc = bacc.Bacc("TRN2", target_bir_lowering=False, debug=False,
                   num_devices=NCORES)

    xT = nc.dram_tensor("xT", [C, R], BF16, kind="ExternalInput").ap()
    # wqk/wv are host-packed to [128, kc*n] so the whole tensor loads in
    # one full-bandwidth DMA (contiguous 2-4KB rows, no sub-512B penalty)
    wqk = nc.dram_tensor("wqk", [128, KC * 2 * LC], BF16,
                         kind="ExternalInput").ap()
    wv = nc.dram_tensor("wv", [128, KC * LC], BF16, kind="ExternalInput").ap()
    bqk = nc.dram_tensor("bqk", [2 * LC], F32, kind="ExternalInput").ap()
    wp = nc.dram_tensor("wp", [LC, C], BF16, kind="ExternalInput").ap()
    trimask = nc.dram_tensor("trimask", [KA, KA], BF16,
                             kind="ExternalInput").ap()
    outT = nc.dram_tensor("outT", [C, R], BF16, kind="ExternalOutput").ap()

    with tile.TileContext(nc) as tc:
        with (
            tc.tile_pool(name="consts", bufs=1) as consts,
            tc.tile_pool(name="weights", bufs=1) as weights,
            tc.tile_pool(name="qkvT", bufs=1) as qkvT_pool,
            tc.tile_pool(name="xs", bufs=NRT) as xs_pool,
            tc.tile_pool(name="vh", bufs=2 * B * T // KA) as vh_pool,
            tc.tile_pool(name="et", bufs=4) as et_pool,
            tc.tile_pool(name="rec", bufs=2) as rec_pool,
            tc.tile_pool(name="osb", bufs=12) as osb_pool,
            tc.tile_pool(name="dscr", bufs=4, space="DRAM") as dscr_pool,
            tc.tile_pool(name="mm512", bufs=2, space="PSUM") as mm512_pool,
            tc.tile_pool(name="ytps", bufs=2, space="PSUM") as ytps_pool,
            tc.tile_pool(name="smps", bufs=2, space="PSUM") as smps_pool,
        ):
            # ---- constants ----
            ones64_f = consts.tile([1, HS], F32)
            nc.vector.memset(ones64_f, 1.0)
            ones64 = consts.tile([1, HS], F32R)
            nc.vector.tensor_copy(ones64, ones64_f)
            tri_sb = consts.tile([KA, KA], BF16)
            bqk_sb = consts.tile([128, 2], F32)

            wqk_sb = weights.tile([128, KC, 2 * LC], BF16)
            wv_sb = weights.tile([128, KC, LC], BF16)
            wp_sb = weights.tile([LC, C], BF16)

            # All x is prefetched up front (no DMA waits inside the
            # attention phase). HWDGE generation (~630ns) and the transfer
            # pipe serialize across DMAs, so keep the COUNT minimal:
            # one DMA per x row tile, whole-tensor weight loads.
            x_tiles = []
            for rt in range(NRT):
                x_sb = xs_pool.tile([128, KC, RT], BF16, tag="xs",
                                    name=f"x_sb{rt}")
                x_tiles.append(x_sb)

            def load_x(rt, lo=0, hi=KC):
                x_r = xT[:, rt * RT:(rt + 1) * RT].rearrange(
                    "(kc p) r -> p kc r", p=128)
                nc.scalar.dma_start(out=x_tiles[rt][:, lo:hi],
                                    in_=x_r[:, lo:hi])

            nc.sync.dma_start(out=_flat(wqk_sb, 0, KC * 2 * LC), in_=wqk)
            load_x(0, 0, 4)
            nc.sync.dma_start(
                out=bqk_sb, in_=bqk.rearrange("(j p) -> p j", p=128))
            load_x(0, 4, KC)
            nc.sync.dma_start(out=_flat(wv_sb, 0, KC * LC), in_=wv)
            load_x(1)
            nc.sync.dma_start(out=tri_sb, in_=trimask)
            nc.sync.dma_start(out=wp_sb, in_=wp)
            for rt in range(2, 5):
                load_x(rt)
            # rt5-7 x loads are deferred into the batch-0 loop so the DMA
            # pipe is clear for batch-0's output stores

            # warmup matmuls: bridge the ~3us DMA startup with PE work so
            # the p-state ramp hits full clock before the first real matmul
            wu_ps = ytps_pool.tile([1, HS], F32, tag="yt", name="wu_ps0")
            for w in range(10):
                if w % 2 == 0 and w > 0:
                    wu_ps = ytps_pool.tile([1, HS], F32, tag="yt",
                                           name=f"wu_ps{w}")
                nc.tensor.matmul(wu_ps, ones64_f[:, 0:1], ones64_f,
                                 start=True, stop=True)

            qT_s = qkvT_pool.tile([LC, R], BF16, tag="qT")
            kT_s = qkvT_pool.tile([LC, R], BF16, tag="kT")
            ynT_s = qkvT_pool.tile([LC, R], BF16, tag="ynT")

            vh_tiles = {}   # (b, chunk) -> [128, 130] tile (65 per head)
            qkv_done = set()  # row tiles whose QKV thunks have all run
            epi_rr = [0]    # epilogue engine round-robin state

            act_free = [True]  # ACT has slack until exps start / after last

            def psum_to_sbuf(dst, src, bias=None):
                """PSUM->SBUF epilogue: DVE while ACT is exp-bound, 50/50
                DVE/ACT otherwise."""
                i = epi_rr[0]
                epi_rr[0] += 1
                on_act = act_free[0] and (i % 2 == 1)
                if bias is not None:
                    if on_act:
                        nc.scalar.activation(
                            dst, src, mybir.ActivationFunctionType.Identity,
                            bias=bias)
                    else:
                        nc.vector.tensor_scalar_add(dst, src, bias)
                    return
                if on_act:
                    nc.scalar.activation(
                        dst, src, mybir.ActivationFunctionType.Identity)
                else:
                    nc.vector.tensor_copy(dst, src)

            def qk_mm(rt, col, lohi, ps):
                dst = qT_s if col == 0 else kT_s
                for kc in range(*lohi):
                    nc.tensor.matmul(
                        ps,
                        wqk_sb[:, kc, col * LC:(col + 1) * LC],
                        x_tiles[rt][:, kc, :],
                        start=(kc == 0),
                        stop=(kc == KC - 1),
                    )
                if lohi[1] == KC:
                    psum_to_sbuf(dst[:, rt * RT:(rt + 1) * RT], ps,
                                 bias=bqk_sb[:, col:col + 1])

            def v_mm(rt, j, ps):
                """v for row chunk (rt*4+j) directly in [r, hs] layout."""
                for kc in range(KC):
                    nc.tensor.matmul(
                        ps[:, j * KA:(j + 1) * KA],
                        x_tiles[rt][:, kc, j * KA:(j + 1) * KA],
                        wv_sb[:, kc, :],
                        start=(kc == 0),
                        stop=(kc == KC - 1),
                    )

            def v_epi(rt, j, ps):
                chunk = rt * (RT // KA) + j
                b = chunk // (T // KA)
                ch = chunk % (T // KA)
                vh = vh_pool.tile([128, 2 * (HS + 1)], BF16, tag="vh",
                                  name=f"vh_b{b}c{ch}")
                src = ps[:, j * KA:(j + 1) * KA]
                nc.vector.tensor_copy(
                    bass.AP(tensor=vh.tensor, offset=vh.offset,
                            ap=[list(vh.ap[0]), [HS + 1, 2], [1, HS]]),
                    bass.AP(tensor=src.tensor, offset=src.offset,
                            ap=[list(src.ap[0]), [HS, 2], [1, HS]]),
                )
                nc.gpsimd.memset(
                    bass.AP(tensor=vh.tensor, offset=vh.offset + HS,
                            ap=[list(vh.ap[0]), [HS + 1, 2], [1, 1]]),
                    1.0)
                vh_tiles[(b, ch)] = vh

            def qkv_rt_thunks(rt):
                """Thunk list for one 512-row QKV tile (~2.5us of PE)."""
                ps_qk = [None, None]
                ps_v = [None]

                def qk_a(col):
                    def f():
                        ps_qk[col] = mm512_pool.tile(
                            [128, RT], F32, tag="mm512",
                            name=f"qkps{rt}c{col}")
                        qk_mm(rt, col, (0, KC // 2), ps_qk[col])
                    return f

                def qk_b(col):
                    return lambda: qk_mm(rt, col, (KC // 2, KC), ps_qk[col])

                def v_a(j):
                    def f():
                        if j == 0:
                            ps_v[0] = mm512_pool.tile(
                                [128, RT], F32, tag="mm512",
                                name=f"vps{rt}")
                        v_mm(rt, j, ps_v[0])
                    return f

                def v_b(j):
                    return lambda: v_epi(rt, j, ps_v[0])

                out = [qk_a(0), qk_b(0), qk_a(1), qk_b(1)]
                for j in range(RT // KA):
                    out.append(v_a(j))
                for j in range(RT // KA):
                    out.append(v_b(j))
                out.append(lambda: qkv_done.add(rt))
                return out

            o2_state = {}

            def proj_thunk(rt, cc):
                """One c_proj output chunk: matmul + epilogue (+store)."""
                def f():
                    pp = mm512_pool.tile([128, RT], F32, tag="mm512",
                                         name=f"pp{rt}c{cc}")
                    nc.tensor.matmul(
                        pp,
                        wp_sb[:, cc * 128:(cc + 1) * 128],
                        ynT_s[:, rt * RT:(rt + 1) * RT],
                        start=True,
                        stop=True,
                    )
                    if cc % 2 == 0:
                        o2_state[rt] = osb_pool.tile(
                            [128, 2, RT], BF16, tag="osb",
                            name=f"o2_{rt}_{cc}")
                    o2 = o2_state[rt]
                    psum_to_sbuf(o2[:, cc % 2], pp)
                    if cc % 2 == 1:
                        dst = outT[(cc - 1) * 128:(cc + 1) * 128,
                                   rt * RT:(rt + 1) * RT]
                        nc.sync.dma_start(
                            out=dst.rearrange("(g p) r -> p g r", p=128),
                            in_=o2)
                return f

            bg = []

            def bg_step(n=1):
                for _ in range(n):
                    if bg:
                        bg.pop(0)()

            def attn_qt(b, h, qt, tail=False):
                # make sure the QKV thunks this tile reads are all emitted
                need_rt = b * (NRT // 2) + qt
                while need_rt not in qkv_done and bg:
                    bg_step(1)
                base = b * T
                hsl = slice(h * HS, (h + 1) * HS)
                qcols = slice(base + qt * QT, base + (qt + 1) * QT)
                nka = (qt + 1) * (QT // KA)
                pairs = [(kc, kc + 1) for kc in range(0, nka, 2)]
                pend = None

                yp = ytps_pool.tile([HS + 1, QT], F32, tag="yt",
                                    name=f"yt_b{b}h{h}q{qt}")

                def av_pair(info):
                    sps2, et2, p2 = info
                    for i, kc in enumerate(p2):
                        diag = (kc * KA // QT == qt)
                        off = kc * KA - qt * QT if diag else 0
                        nc.tensor.matmul(
                            yp[:, off:QT],
                            vh_tiles[(b, kc)][:, h * (HS + 1):
                                              (h + 1) * (HS + 1)],
                            et2[:, i, off:QT],
                            start=(kc == 0),
                            stop=(kc == nka - 1),
                        )

                for p2 in pairs:
                    sps2 = smps_pool.tile(
                        [128, 2, QT], F32, tag="sm",
                        name=f"sps_b{b}h{h}q{qt}k{p2[0]}")
                    for i, kc in enumerate(p2):
                        nc.tensor.matmul(
                            sps2[:, i],
                            kT_s[hsl, base + kc * KA:base + (kc + 1) * KA],
                            qT_s[hsl, qcols],
                            start=True,
                            stop=True,
                        )
                    et2 = et_pool.tile([128, 2, QT], BF16, tag="et",
                                       name=f"et_b{b}h{h}q{qt}k{p2[0]}")
                    offs = [kc * KA - qt * QT if kc * KA // QT == qt else 0
                            for kc in p2]
                    scale = 1.0 / np.sqrt(HS).item()
                    if offs[1] <= KA:
                        # one flat span; <=128 wasted rows beat a 2nd init
                        nc.scalar.activation(
                            _flat(et2, offs[0], 2 * QT),
                            _flat(sps2, offs[0], 2 * QT),
                            mybir.ActivationFunctionType.Exp, scale=scale)
                    else:
                        for i in range(2):
                            nc.scalar.activation(
                                _flat(et2, i * QT + offs[i], (i + 1) * QT),
                                _flat(sps2, i * QT + offs[i], (i + 1) * QT),
                                mybir.ActivationFunctionType.Exp,
                                scale=scale)
                    for i, kc in enumerate(p2):
                        if kc * KA // QT == qt:
                            # Pool: contention-free (DVE queueing would put
                            # epilogues on the exp->mask->AV chain)
                            o = kc * KA - qt * QT
                            nc.gpsimd.tensor_mul(
                                et2[:, i, o:o + KA], et2[:, i, o:o + KA],
                                tri_sb)
                    if pend is not None:
                        av_pair(pend)
                    pend = (sps2, et2, p2)
                    bg_step(2 if len(bg) > 12 else 1)
                av_pair(pend)

                # softmax normalize. Default: DMA-bounce broadcast of the
                # reciprocal row (no PE/ACT cost, ~3.5us latency hidden by
                # the next qt). Tail: rank-1 PE broadcast + SBUF bounce of
                # y, the shortest-latency chain.
                rec = rec_pool.tile([1, QT], F32R, tag="rec",
                                    name=f"rec_b{b}h{h}q{qt}")
                with nc.allow_low_precision(
                        reason="f32r reciprocal: ~1e-4 rel err ok"):
                    nc.vector.reciprocal(rec, yp[HS:HS + 1, :])
                if tail or b == 0:
                    # batch 0: the x prefetch owns the DMA pipe (FIFO), a
                    # bounce would queue ~20us behind it -- use the PE
                    # rank-1 broadcast instead. Also the tail (latency).
                    yc = rec_pool.tile([HS, QT], F32, tag="yc",
                                       name=f"yc_b{b}h{h}q{qt}")
                    nc.scalar.activation(
                        yc, yp[0:HS, :],
                        mybir.ActivationFunctionType.Identity)
                    bcp = mm512_pool.tile([HS, QT], F32, tag="mm512",
                                          name=f"bcp_b{b}h{h}q{qt}")
                    nc.tensor.matmul(bcp, ones64, rec, start=True, stop=True)
                    nc.vector.tensor_mul(ynT_s[hsl, qcols], yc, bcp)
                else:
                    recd = dscr_pool.tile([1, QT], F32R, tag="recd",
                                          name=f"recd_b{b}h{h}q{qt}")
                    nc.sync.dma_start(out=recd, in_=rec)
                    bcs = rec_pool.tile([HS, QT], F32R, tag="bcs",
                                        name=f"bcs_b{b}h{h}q{qt}")
                    rec_bcast = bass.AP(
                        tensor=recd.tensor, offset=recd.offset,
                        ap=[[0, HS]] + [list(d) for d in recd.ap[1:]])
                    nc.sync.dma_start(out=bcs, in_=rec_bcast)
                    nc.vector.tensor_mul(ynT_s[hsl, qcols], yp[0:HS, :], bcs)
                bg_step(2)

            # ---- symmetric per-batch schedule: first row tile direct,
            # then per qt: both heads' attention with the next row tile's
            # QKV as background filler, proj chasing one qt behind ----
            pend_proj = []
            for b in range(B):
                rt0 = b * (NRT // 2)
                bg.extend(pend_proj)
                pend_proj = []
                for th in qkv_rt_thunks(rt0):
                    th()
                for qt in range(NQT):
                    if b == 0 and qt < 3:
                        load_x(5 + qt)  # deferred b1 x prefetch
                    if qt + 1 < NQT:
                        bg.extend(qkv_rt_thunks(rt0 + qt + 1))
                    act_free[0] = False
                    attn_qt(b, 0, qt)
                    # previous qt's proj joins bg only now: its ynT
                    # multiply needs a full sub-phase of slack
                    bg.extend(pend_proj)
                    pend_proj = []
                    attn_qt(b, 1, qt, tail=(b == B - 1 and qt == NQT - 1))
                    pend_proj = [proj_thunk(rt0 + qt, cc)
                                 for cc in range(NCC)]
            bg.extend(pend_proj)
            # exps are done: the drain's epilogues can use ACT again
            act_free[0] = True
            while bg:
                bg_step(1)

    nc.compile()
    return nc


_NC = None


def _get_nc():
    global _NC
    if _NC is None:
        _NC = build_program()
    return _NC


def make_in_maps(x, W_attn, b_attn, W_proj, b_proj):
    x = np.asarray(x, np.float32)
    W_attn = np.asarray(W_attn, np.float32)
    b_attn = np.asarray(b_attn, np.float32)
    W_proj = np.asarray(W_proj, np.float32)

    xT = np.ascontiguousarray(x.reshape(R, C).T).astype(BF)
    tri = np.triu(np.ones((KA, KA), np.float32)).astype(BF)

    in_maps = []
    for core in range(NCORES):
        g0 = core * LC
        cols = slice(g0, g0 + LC)
        wqk_l = np.concatenate(
            [W_attn[:, 0:C][:, cols], W_attn[:, C:2 * C][:, cols]], axis=1)
        bqk_l = np.concatenate(
            [b_attn[0:C][cols], b_attn[C:2 * C][cols]])
        wv_l = W_attn[:, 2 * C:3 * C][:, cols]

        def pack(w):  # [C, n] -> [128, KC*n]: one contiguous DMA per core
            n = w.shape[1]
            return np.ascontiguousarray(
                w.reshape(KC, 128, n).transpose(1, 0, 2).reshape(128, -1))
        in_maps.append({
            "xT": xT,
            "wqk": pack(wqk_l).astype(BF),
            "wv": pack(wv_l).astype(BF),
            "bqk": np.ascontiguousarray(bqk_l),
            "wp": np.ascontiguousarray(W_proj[cols, :]).astype(BF),
            "trimask": tri,
        })
    return in_maps


def kernel(x, W_attn, b_attn, W_proj, b_proj):
    nc = _get_nc()
    in_maps = make_in_maps(x, W_attn, b_attn, W_proj, b_proj)
    res = run_bass_kernel_spmd(nc, in_maps, list(range(NCORES)))
    acc = res.results[0]["outT"].astype(np.float32)
    for corer in res.results[1:]:
        acc += corer["outT"].astype(np.float32)
    out = np.ascontiguousarray(acc.T).reshape(B, T, C)
    # v-bias and c_proj bias fold into the host-side reduction epilogue:
    # softmax rows sum to 1, so b_v contributes b_v @ W_proj to every row.
    b_attn = np.asarray(b_attn, np.float32)
    out += np.asarray(b_proj, np.float32) + b_attn[2 * C:] @ np.asarray(
        W_proj, np.float32)
    return out
